# revision 21
# baseline (speedup 1.0000x reference)
"""SE(3)-CNN block (TensorProduct -> SE3Conv -> SE3BatchNorm -> BiasRelu) on 8 trn2 cores.

Sharding: core c = (batch b=c//2, out-x-half h=c%2). Each core computes all 64
output channels for 8 of 16 output x-planes of one batch; per-field BN second
moments are combined with a tiny [1,64] AllReduce across all 8 cores.

Conv strategy: the 9 t-channels per vector pair are symmetric (t = v (x) v), so
the 208 input channels reduce to 160 symmetrized ones. The contraction runs as
fp32r matmuls, one per (ky, kz, kx-pair, psum-bank), with free dim spanning TWO
output-x planes (2 x oyc x 16 <= 512 = one full psum bank) so each instruction
streams ~484 elements. kx tap pairs (ka, ka+2) share one rhs read: lhsT cols
0:64 = tap ka (accumulates plane (px-ka)/2), cols 64:128 = tap ka+2 (plane one
lower), using the slot trick: psum slot s holds plane s in partitions 0:64 and
plane s-1 in partitions 64:128. Slot s = (bank s//2, half s%2); a double-slot
matmul covers slots (2m, 2m+1) = bank m. Chunk1 (channels 0:128) uses slots
0..8 (banks 0-4); chunk2 (channels 128:160, stored as 4 x-shifted copies so 4
kx taps pack into 128 contraction rows) uses slots 10..15 (banks 5-7) plus
edge writes into chunk1 slots. Chunk2 runs first so its psum banks finish ~60us
in and evacuate overlapped with chunk1 matmuls.
"""
import numpy as np
from itertools import product

# problem constants (from spec / reference)
B = 4
S_IN = 16
V_IN = 16
CO = 64          # 16 scalar + 48 vector output channels
CI = 160         # 16 s + 48 v + 96 t_sym
SIZE = 7
PAD = 3
STRIDE = 2
EPS = 1e-5
NCORES = 8
NXS = 21         # x-padded slab planes per core (px 0..20 read)
NXS2 = 10        # chunk-2 half-x slab planes (px = 0..18 even)
NZS = 19         # z-padded: zi_slab = zi_global + 2, covering zofs in [-2, 1]
OXC = 8          # out x-planes per core
PAIRS = [(0, 0), (0, 1), (0, 2), (1, 1), (1, 2), (2, 2)]
VAR_S_DIV = 1.0 / (B * 16 * 16 * 16)
VAR_V_DIV = 1.0 / (B * 3 * 16 * 16 * 16)

SLAB_SHAPE = (128, NXS, 32, 2, NZS)    # [ci, px, iy, pz, zi]
SLAB2_SHAPE = (128, NXS2, 32, 2, NZS)  # [4x32 shifted c2, xi=px/2, iy, pz, zi]
WA_COLS = 448   # 7 single-tap blocks: pair cols [k0|k2][k1|k3][k4|k6][k5]
W2_COLS = 128   # [g0: kx=a | g1: kx=4+a] for row block a

KX_PAIRS = [(0, 2), (1, 3), (4, 6)]
WC_TAP = {0: 0, 2: 64, 1: 128, 3: 192, 4: 256, 6: 320, 5: 384}


# ---------------------------------------------------------------- host prep

def _assemble_kernel_sym(inp):
    """Assemble the dense conv kernel [64, 208, 7,7,7] and symmetrize the
    t-block -> [64, 160, 7,7,7]."""
    def blk(w, basis):
        w = np.asarray(w, np.float32)
        basis = np.asarray(basis, np.float32)
        mo, mi, nb = w.shape
        do, di = basis.shape[1], basis.shape[2]
        k = np.einsum('uvb,bijxyz->uivjxyz', w, basis)
        return k.reshape(mo * do, mi * di, SIZE, SIZE, SIZE)

    row_s = np.concatenate([blk(inp['w_ss'], inp['basis_ss']),
                            blk(inp['w_sv'], inp['basis_sv']),
                            blk(inp['w_st'], inp['basis_st'])], axis=1)
    row_v = np.concatenate([blk(inp['w_vs'], inp['basis_vs']),
                            blk(inp['w_vv'], inp['basis_vv']),
                            blk(inp['w_vt'], inp['basis_vt'])], axis=1)
    K = np.concatenate([row_s, row_v], axis=0)  # [64, 208, 7,7,7]

    Ks = np.empty((CO, CI, SIZE, SIZE, SIZE), np.float32)
    Ks[:, :64] = K[:, :64]
    for u in range(16):
        for pi, (i, j) in enumerate(PAIRS):
            src = K[:, 64 + 9 * u + 3 * i + j]
            if i != j:
                src = src + K[:, 64 + 9 * u + 3 * j + i]
            Ks[:, 64 + 6 * u + pi] = src
    return Ks


def _svt_sym(sv):
    """[4,64,32,32,32] -> symmetrized tensor-product features [4,160,32,32,32]."""
    sv = np.asarray(sv, np.float32)
    s = sv[:, :S_IN]
    v = sv[:, S_IN:].reshape(B, V_IN, 3, 32, 32, 32)
    t = np.empty((B, V_IN, 6, 32, 32, 32), np.float32)
    for pi, (i, j) in enumerate(PAIRS):
        t[:, :, pi] = v[:, :, i] * v[:, :, j]
    return np.concatenate([s, v.reshape(B, 48, 32, 32, 32),
                           t.reshape(B, 96, 32, 32, 32)], axis=1)


def _core_slabs(svt, b, h):
    """x/z zero-padded, z-parity-split slabs for core (b, h).

    Returns (c1, c2e): c1 SLAB_SHAPE, plane px holds global ix = px + 16h - 3;
    c2e SLAB2_SHAPE, block a (rows 32a:32a+32) of plane xi holds chunk-2
    channels at ix = 2*xi + a + 16h - 3. zi_slab = zi_global + 2.
    """
    sp = svt[b].reshape(CI, 32, 32, 16, 2)   # (ci, x, y, zi, pz); iz = 2*zi + pz
    sp = np.moveaxis(sp, 4, 3)               # (ci, x, y, pz, zi)
    x0 = 16 * h - 3
    c1 = np.zeros(SLAB_SHAPE, np.float32)
    lo, hi = max(0, x0), min(32, x0 + NXS)
    c1[:, lo - x0:hi - x0, :, :, 2:18] = sp[:128, lo:hi]
    c2 = np.zeros(SLAB2_SHAPE, np.float32)
    for a in range(4):
        for xi in range(NXS2):
            ix = 2 * xi + a + x0
            if 0 <= ix < 32:
                c2[32 * a:32 * a + 32, xi, :, :, 2:18] = sp[128:160, ix]
    return c1, c2


def _weight_slabs(Ks):
    """(WA, W2): chunk-1 tap blocks [49, 128, 448] and the 4-way kx-merged
    chunk-2 slab [49, 128, 128]."""
    WA = np.zeros((49, 128, WA_COLS), np.float32)
    W2 = np.zeros((49, 128, W2_COLS), np.float32)
    for ky, kz in product(range(SIZE), range(SIZE)):
        i = ky * SIZE + kz
        for kx, off in WC_TAP.items():
            WA[i, :, off:off + 64] = Ks[:, :128, kx, ky, kz].T
        for g in range(2):
            for a in range(4):
                kx = 4 * g + a
                if kx > 6:
                    continue
                W2[i, 32 * a:32 * a + 32, 64 * g:64 * (g + 1)] = \
                    Ks[:, 128:160, kx, ky, kz].T
    return WA, W2


def _gam_bias(bn_g_s, bn_g_v, bias_s):
    """Per-channel gamma [64] (vector gammas replicated x3) and bias [64]."""
    gam = np.empty(64, np.float32)
    gam[:16] = np.asarray(bn_g_s, np.float32)
    gam[16:] = np.repeat(np.asarray(bn_g_v, np.float32), 3)
    bias = np.zeros(64, np.float32)
    bias[:16] = np.asarray(bias_s, np.float32)
    return gam, bias


def _aux_mat():
    """[128, 128] constant: cols 0:64 on rows 0:64 = variance-combining
    matrix C (vloc = C.T @ ssq); cols 64:128 on rows 64:128 = identity (the
    partition-shift matmul that moves upper-half psum partials down)."""
    aux = np.zeros((128, 128), np.float32)
    for c in range(16):
        aux[c, c] = VAR_S_DIV
    for c in range(16, 64):
        g = (c - 16) // 3
        for j in range(3):
            aux[16 + 3 * g + j, c] = VAR_V_DIV
    aux[64:, 64:] = np.eye(64, dtype=np.float32)
    return aux


# ---------------------------------------------------------------- matmul plan

def _box(ky, kz):
    """Valid output range + slab coords for kernel offsets (ky, kz)."""
    d = kz - 3
    p = d % 2
    zofs = (d - p) // 2
    oy0 = max(0, (4 - ky) // 2)
    oy1 = min(16, (34 - ky) // 2 + 1)
    iy0 = 2 * oy0 + ky - 3
    return dict(p=p, zs=zofs + 2, iy0=iy0, oyc=oy1 - oy0, oy0=oy0)


def _mm_plan():
    """Matmul descriptors in issue order.

    Each entry: (i, slab, xi0, nx, wc, ww, bank, h0, nh, p0, p1) where
    slab 2 reads sl2[:, xi0:xi0+nx] (stride 1), slab 1 reads
    sl1[:, xi0:xi0+2*nx:2]; lhsT = W{a,2}[i][:, wc:wc+ww]; out =
    pq[bank][p0:p1, h0:h0+nh, oy...]. Chunk-2 phase first (its psum banks
    finish early and its compute covers the chunk-1 slab's DMA).
    """
    plan = []
    for i in range(49):
        plan.append((i, 2, 0, 2, 448 + 0, 64, 0, 0, 2, 0, 64))    # c2s g0 -> planes 0,1 L
        for q in range(3):                                         # c2 full
            plan.append((i, 2, 2 + 2 * q, 2, 448, 128, 5 + q, 0, 2, 0, 128))
        plan.append((i, 2, 8, 2, 448 + 64, 64, 3, 0, 2, 0, 64))    # c2s g1 -> planes 6,7 L
    for i in range(49):
        for ka, kb in KX_PAIRS:
            wc = WC_TAP[ka]
            assert WC_TAP[kb] == wc + 64
            for m in range(4):
                plan.append((i, 1, ka + 4 * m, 2, wc, 128, m, 0, 2, 0, 128))
            # edge: plane 7 via tap kb at px = ka+16 -> slot 8 LOWER (psum
            # writes at partition offset 64 fail the fp32r ISA check)
            plan.append((i, 1, ka + 16, 1, wc + 64, 64, 4, 0, 1, 0, 64))
        for m in range(4):                                         # single tap 5
            plan.append((i, 1, 5 + 4 * m, 2, WC_TAP[5], 64, m, 0, 2, 0, 64))
    return plan


def _regions(d):
    """(slot, 'L'/'U') psum regions written by descriptor d."""
    _, _, _, _, _, _, bank, h0, nh, p0, p1 = d
    out = []
    for dh in range(nh):
        s = 2 * bank + h0 + dh
        if p0 == 0:
            out.append((s, 'L'))
        if p1 == 128:
            out.append((s, 'U'))
    return out


_PLAN = _mm_plan()
_LAST_IDX = {}
for _n, _d in enumerate(_PLAN):
    for _r in _regions(_d):
        _LAST_IDX[_r] = _n
_STOPS = set(_LAST_IDX.values())
N_C2 = 49 * 5


# ---------------------------------------------------------------- numpy shadow

def _shadow_core(c1, c2, WA, W2):
    """Execute the matmul plan in numpy. Returns conv output [64, 8, 16, 16]."""
    # psum: [bank, half, part, y, z]
    ps = np.zeros((8, 2, 128, 16, 16), np.float32)
    for i, slab, xi0, nx, wc, ww, bank, h0, nh, p0, p1 in _PLAN:
        ky, kz = divmod(i, 7)
        bx = _box(ky, kz)
        W = W2 if wc >= 448 else WA
        lhsT = W[i][:, (wc - 448 if wc >= 448 else wc):][:, :ww]
        sl = c2 if slab == 2 else c1
        step = 1 if slab == 2 else 2
        for dh in range(nh):
            xi = xi0 + step * dh
            rhs = sl[:, xi, bx['iy0']:bx['iy0'] + 2 * bx['oyc']:2, bx['p'],
                     bx['zs']:bx['zs'] + 16]
            contrib = np.einsum('km,kbc->mbc', lhsT, rhs)
            ys = slice(bx['oy0'], bx['oy0'] + bx['oyc'])
            ps[bank, h0 + dh, p0:p1, ys, :] += contrib
    # slot s = (bank s//2, half s%2); L = parts 0:64, U = 64:128
    def L(s):
        return ps[s // 2, s % 2, 0:64]

    def U(s):
        return ps[s // 2, s % 2, 64:128]

    out = np.empty((OXC, CO, 16, 16), np.float32)
    for j in range(OXC):
        out[j] = L(j) + (U(j + 1) if j <= 6 else L(8))
        if j >= 2:
            out[j] += L(10 + (j - 2))
        if j <= 5:
            out[j] += U(10 + j)
    return out.transpose(1, 0, 2, 3)


def shadow_forward(inp):
    """Full-model numpy shadow of the device computation (for plan validation)."""
    svt = _svt_sym(inp['sv'])
    Ks = _assemble_kernel_sym(inp)
    WA, W2 = _weight_slabs(Ks)
    gam, bias = _gam_bias(inp['bn_g_s'], inp['bn_g_v'], inp['bias_s'])

    y = np.zeros((B, CO, 16, 16, 16), np.float32)
    ss = np.zeros(64, np.float64)
    for c in range(NCORES):
        b, h = c // 2, c % 2
        c1, c2 = _core_slabs(svt, b, h)
        out = _shadow_core(c1, c2, WA, W2)
        y[b, :, 8 * h:8 * h + 8] = out
        ss += (out.astype(np.float64) ** 2).sum(axis=(1, 2, 3))

    var = np.empty(64)
    var[:16] = ss[:16] * VAR_S_DIV
    vv = (ss[16::3] + ss[17::3] + ss[18::3]) * VAR_V_DIV
    var[16:] = np.repeat(vv, 3)
    scale = gam / np.sqrt(var + EPS)
    y = y * scale[None, :, None, None, None].astype(np.float32)
    y[:, :16] = np.maximum(y[:, :16] + bias[:16][None, :, None, None, None], 0.0)
    return y


# ---------------------------------------------------------------- bass kernel

_CACHED = {}


def _build_bass():
    import concourse.bass as bass
    import concourse.tile as tile
    import concourse.mybir as mybir
    from concourse import bacc

    f32 = mybir.dt.float32
    f32r = mybir.dt.float32r

    nc = bacc.Bacc("TRN2", target_bir_lowering=False, debug=False, num_devices=NCORES)

    in1 = nc.dram_tensor("in1", list(SLAB_SHAPE), f32r, kind="ExternalInput").ap()
    in2 = nc.dram_tensor("in2", list(SLAB2_SHAPE), f32r, kind="ExternalInput").ap()
    wa_in = nc.dram_tensor("wa_in", [49, 128, WA_COLS], f32r, kind="ExternalInput").ap()
    w2_in = nc.dram_tensor("w2_in", [128, 49 * W2_COLS], f32r, kind="ExternalInput").ap()
    aux_in = nc.dram_tensor("aux_in", [128, 128], f32, kind="ExternalInput").ap()
    gam_in = nc.dram_tensor("gam_in", [64, 1], f32, kind="ExternalInput").ap()
    bias_in = nc.dram_tensor("bias_in", [64, 1], f32, kind="ExternalInput").ap()
    out_d = nc.dram_tensor("out", [CO, OXC, 16, 16], f32, kind="ExternalOutput").ap()

    with tile.TileContext(nc) as tc:
        with (
            tc.tile_pool(name="slab", bufs=1) as slab_pool,
            tc.tile_pool(name="wpa", bufs=6) as wpa_pool,
            tc.tile_pool(name="ps", bufs=1, space="PSUM") as ps,
            tc.tile_pool(name="outp", bufs=1) as outp,
            tc.tile_pool(name="stat", bufs=1) as stat,
            tc.tile_pool(name="dram", bufs=1, space="DRAM") as dram,
        ):
            # 8 psum banks = 16 half-bank slots
            pq = [ps.tile([128, 2, 16, 16], f32, tag=f"pq{t}", name=f"pq{t}")
                  for t in range(8)]

            # inputs balanced across the two HWDGE queues: scalar gets sl2
            # (gates the chunk-2 phase plane-by-plane) then the sl1 tail;
            # sync gets the sl1 head. Each queue moves ~10MB (~45us).
            sl1 = slab_pool.tile(list(SLAB_SHAPE), f32r, tag="slab", name="slab_c1")
            sl2 = slab_pool.tile(list(SLAB2_SHAPE), f32r, tag="slab2",
                                 name="slab_c2")
            for xi in range(NXS2):
                nc.scalar.dma_start(sl2[:, xi], in2[:, xi])

            # HBM priority order: the chunk-2 phase's inputs (w2, sl2) go
            # FIRST in the shared HBM pipe so the tensor engine isn't idle
            # during the big sl1/wa load. w2 lives in one [128, 49*128] tile
            # (partition-major DRAM layout) loaded in 4 chunked DMAs.
            aux_t = stat.tile([128, 128], f32, tag="aux")
            w2s = slab_pool.tile([128, 49 * W2_COLS], f32r, tag="w2s",
                                 name="w2s")
            wat = [wpa_pool.tile([128, WA_COLS], f32r, tag="wa", name=f"wa_{i}")
                   for i in range(49)]
            for c0, c1 in ((0, 1664), (1664, 3328), (3328, 4992), (4992, 6272)):
                nc.sync.dma_start(w2s[:, c0:c1], w2_in[:, c0:c1])
            nc.sync.dma_start(aux_t[:], aux_in[:])
            for px in range(14):
                nc.sync.dma_start(sl1[:, px], in1[:, px])
            for px in range(14, NXS):
                nc.scalar.dma_start(sl1[:, px], in1[:, px])
            for i in range(49):
                nc.sync.dma_start(wat[i][:], wa_in[i])

            # start=True clears the WHOLE psum bank, so open each bank once
            # with a zero-weight full-bank matmul (also a WAW dep that orders
            # it before every accumulate); all real matmuls use start=False.
            # The rhs is our own memset tile so openers don't wait on input
            # DMAs.
            zw_f = stat.tile([128, 512], f32, tag="zw")
            nc.vector.memset(zw_f[:], 0.0)
            zw = zw_f.bitcast(f32r)
            for t in range(8):
                nc.tensor.matmul(pq[t].rearrange("c a y z -> c (a y z)"),
                                 zw[:, 0:128], zw[:, :], start=True, stop=False)

            # warm the sqrt/square/relu activation table on the scalar engine
            # now so the tail's activations don't pay the 1.3us table load
            eps_t = stat.tile([CO, 1], f32, tag="eps")
            nc.vector.memset(eps_t[:], EPS)
            sd = stat.tile([CO, 1], f32, tag="sd")
            nc.scalar.activation(sd[0:1], eps_t[0:1],
                                 mybir.ActivationFunctionType.Sqrt)

            def emit(n, d):
                i, slab, xi0, nx, wc, ww, bank, h0, nh, p0, p1 = d
                ky, kz = divmod(i, 7)
                bx = _box(ky, kz)
                if wc >= 448:
                    w = w2s
                    wc = W2_COLS * i + (wc - 448)
                else:
                    w = wat[i]
                if slab == 2:
                    rhs = sl2[:, xi0:xi0 + nx, bx['iy0']:bx['iy0'] + 2 * bx['oyc'] - 1:2,
                              bx['p'], bx['zs']:bx['zs'] + 16]
                else:
                    rhs = sl1[:, xi0:xi0 + 2 * nx - 1:2,
                              bx['iy0']:bx['iy0'] + 2 * bx['oyc'] - 1:2,
                              bx['p'], bx['zs']:bx['zs'] + 16]
                out_ap = pq[bank][p0:p1, h0:h0 + nh, bx['oy0']:bx['oy0'] + bx['oyc'], :]
                nc.tensor.matmul(out_ap, w[:, wc:wc + ww], rhs,
                                 start=False, stop=n in _STOPS)

            for n in range(N_C2):
                emit(n, _PLAN[n])

            # early evacuation of chunk2-only banks 5-7, overlapped with the
            # chunk-1 matmuls: LB -> osb planes 2..7, UB -> usb planes 0..5
            osb = outp.tile([CO, OXC, 16, 16], f32, tag="osb")
            usb = outp.tile([128, OXC, 16, 16], f32, tag="usb")
            for q in range(3):
                nc.vector.tensor_copy(osb[:, 2 + 2 * q:4 + 2 * q], pq[5 + q][0:64])
                nc.vector.tensor_copy(usb[64:128, 2 * q:2 * q + 2], pq[5 + q][64:128])

            for n in range(N_C2, len(_PLAN)):
                emit(n, _PLAN[n])

            # late evacuation: plane j needs L(j) [slots 0..7], U(j+1) for
            # j<=6 [slots 1..7], and plane 7 the edge partials from slot 8 L
            nc.vector.tensor_copy(osb[:, 0:2], pq[0][0:64])
            for m in range(1, 4):
                nc.vector.tensor_add(osb[:, 2 * m:2 * m + 2], osb[:, 2 * m:2 * m + 2],
                                     pq[m][0:64])
            nc.vector.tensor_add(osb[:, 7], osb[:, 7], pq[4][0:64, 0])
            nc.vector.tensor_add(usb[64:128, 0], usb[64:128, 0], pq[0][64:128, 1])
            nc.vector.tensor_add(usb[64:128, 1:3], usb[64:128, 1:3], pq[1][64:128])
            nc.vector.tensor_add(usb[64:128, 3:5], usb[64:128, 3:5], pq[2][64:128])
            nc.vector.tensor_add(usb[64:128, 5], usb[64:128, 5], pq[3][64:128, 0])
            nc.vector.tensor_copy(usb[64:128, 6], pq[3][64:128, 1])

            # move upper-half partials (planes 0..6) down to partitions 0:64
            # via identity matmuls through the (now free) psum banks 0-3 —
            # no DRAM round-trip — then add into osb
            ident = aux_t[64:128, 64:128]
            for m in range(3):
                nc.tensor.matmul(pq[m][0:64], ident,
                                 usb[64:128, 2 * m:2 * m + 2],
                                 start=True, stop=True)
            nc.tensor.matmul(pq[3][0:64, 0], ident, usb[64:128, 6],
                             start=True, stop=True)
            for m in range(3):
                nc.vector.tensor_add(osb[:, 2 * m:2 * m + 2],
                                     osb[:, 2 * m:2 * m + 2], pq[m][0:64])
            nc.vector.tensor_add(osb[:, 6], osb[:, 6], pq[3][0:64, 0])

            # per-channel sum of squares in one fused scalar-engine op
            # (square + per-partition accumulate), then one matmul with the
            # combining matrix C turns it into the local variance [64, 1]
            of = osb.rearrange("c x y z -> c (x y z)")
            sq = usb.rearrange("c x y z -> c (x y z)")[0:64, :]
            ssq = stat.tile([CO, 1], f32, tag="ssq")
            nc.scalar.activation(sq[:, :], of[:, :],
                                 mybir.ActivationFunctionType.Square,
                                 accum_out=ssq[:])
            nc.tensor.matmul(pq[5][0:64, 0, 0, 0:1], aux_t[0:64, 0:64],
                             ssq[:, :], start=True, stop=True)
            vcol = stat.tile([CO, 1], f32, tag="vcol")
            nc.vector.tensor_copy(vcol[:], pq[5][0:64, 0, 0, 0:1])

            v_dram = dram.tile([1, 64], f32, tag="vd")
            v_red = dram.tile([1, 64], f32, tag="vr")
            nc.sync.dma_start(v_dram[0, :], vcol[:, 0])
            nc.gpsimd.collective_compute(
                "AllReduce", mybir.AluOpType.add,
                replica_groups=[list(range(NCORES))],
                ins=[v_dram.opt()], outs=[v_red.opt()],
            )

            # scale = gamma / sqrt(var + eps), in per-partition layout
            var_col = stat.tile([CO, 1], f32, tag="varcol")
            nc.sync.dma_start(var_col[:, 0], v_red[0, :])
            nc.scalar.activation(sd[:], var_col[:], mybir.ActivationFunctionType.Sqrt,
                                 bias=eps_t[:], scale=1.0)
            inv = stat.tile([CO, 1], f32, tag="inv")
            nc.vector.reciprocal(inv[:], sd[:])
            gam_t = stat.tile([CO, 1], f32, tag="gam")
            nc.sync.dma_start(gam_t[:], gam_in[:])
            scale_col = stat.tile([CO, 1], f32, tag="sccol")
            nc.vector.tensor_mul(scale_col[:], inv[:], gam_t[:])
            bias_t = stat.tile([CO, 1], f32, tag="bias")
            nc.sync.dma_start(bias_t[:], bias_in[:])

            # BN scale everywhere (DVE partition ranges must be 32-aligned,
            # so no scalar/vector split), then bias+relu on scalar channels;
            # the vector-channel store only waits on the scale
            nc.vector.tensor_scalar_mul(of[:, :], of[:, :], scale_col[:, :])
            nc.scalar.activation(of[0:16, :], of[0:16, :],
                                 mybir.ActivationFunctionType.Relu,
                                 bias=bias_t[0:16, :], scale=1.0)
            # vector-channel store needs only the scale; it rides sync so it
            # overlaps the scalar-engine relu, whose store follows on scalar
            nc.sync.dma_start(out_d[16:64], osb[16:64])
            nc.scalar.dma_start(out_d[0:16], osb[0:16])

    nc.compile()
    return nc


def _install_ntff_hook():
    import sys, types
    if "antenv.axon_hooks" in sys.modules:
        return
    mod = types.ModuleType("antenv.axon_hooks")
    mod._hook = None
    mod.set_axon_ntff_profile_hook = lambda h: setattr(mod, "_hook", h)
    mod.get_axon_ntff_profile_hook = lambda: mod._hook
    sys.modules["antenv.axon_hooks"] = mod
    try:
        import antenv
        antenv.axon_hooks = mod
        from trn_agent_boot.trn_boot import _ntff_profile_via_ctypes
        mod.set_axon_ntff_profile_hook(_ntff_profile_via_ctypes("/opt/axon/libaxon_pjrt.so"))
    except Exception:
        pass


def run_on_hw(inp, trace=False):
    """Run the kernel on 8 cores. Returns (full output [4,64,16,16,16], results)."""
    from concourse.bass_utils import run_bass_kernel_spmd

    if "nc" not in _CACHED:
        _install_ntff_hook()
        _CACHED["nc"] = _build_bass()
    nc = _CACHED["nc"]

    svt = _svt_sym(inp['sv'])
    Ks = _assemble_kernel_sym(inp)
    WA, W2 = _weight_slabs(Ks)
    gam, bias = _gam_bias(inp['bn_g_s'], inp['bn_g_v'], inp['bias_s'])

    in_maps = []
    for c in range(NCORES):
        b, h = c // 2, c % 2
        c1, c2 = _core_slabs(svt, b, h)
        in_maps.append({
            "in1": c1,
            "in2": c2,
            "wa_in": WA,
            "w2_in": np.ascontiguousarray(W2.transpose(1, 0, 2).reshape(128, 49 * W2_COLS)),
            "aux_in": _aux_mat(),
            "gam_in": gam.reshape(64, 1),
            "bias_in": bias.reshape(64, 1),
        })

    res = run_bass_kernel_spmd(nc, in_maps, core_ids=list(range(NCORES)), trace=trace)

    y = np.zeros((B, CO, 16, 16, 16), np.float32)
    for c in range(NCORES):
        b, h = c // 2, c % 2
        y[b, :, 8 * h:8 * h + 8] = res.results[c]["out"]
    return y, res


def kernel(**inputs) -> np.ndarray:
    y, _ = run_on_hw(inputs, trace=False)
    return y


# revision 22
# speedup vs baseline: 1.0640x; 1.0640x over previous
"""SE(3)-CNN block (TensorProduct -> SE3Conv -> SE3BatchNorm -> BiasRelu) on 8 trn2 cores.

Sharding: core c = (batch b=c//2, out-x-half h=c%2). Each core computes all 64
output channels for 8 of 16 output x-planes of one batch; per-field BN second
moments are combined with a tiny [1,64] AllReduce across all 8 cores.

Conv strategy: the 9 t-channels per vector pair are symmetric (t = v (x) v), so
the 208 input channels reduce to 160 symmetrized ones. The contraction runs as
fp32r matmuls, one per (ky, kz, kx-pair, psum-bank), with free dim spanning TWO
output-x planes (2 x oyc x 16 <= 512 = one full psum bank) so each instruction
streams ~484 elements. kx tap pairs (ka, ka+2) share one rhs read: lhsT cols
0:64 = tap ka (accumulates plane (px-ka)/2), cols 64:128 = tap ka+2 (plane one
lower), using the slot trick: psum slot s holds plane s in partitions 0:64 and
plane s-1 in partitions 64:128. Slot s = (bank s//2, half s%2); a double-slot
matmul covers slots (2m, 2m+1) = bank m. Chunk1 (channels 0:128) uses slots
0..8 (banks 0-4); chunk2 (channels 128:160, stored as 4 x-shifted copies so 4
kx taps pack into 128 contraction rows) uses slots 10..15 (banks 5-7) plus
edge writes into chunk1 slots. Chunk2 runs first so its psum banks finish ~60us
in and evacuate overlapped with chunk1 matmuls.
"""
import numpy as np
from itertools import product

# problem constants (from spec / reference)
B = 4
S_IN = 16
V_IN = 16
CO = 64          # 16 scalar + 48 vector output channels
CI = 160         # 16 s + 48 v + 96 t_sym
SIZE = 7
PAD = 3
STRIDE = 2
EPS = 1e-5
NCORES = 8
NXS = 21         # x-padded slab planes per core (px 0..20 read)
NXS2 = 10        # chunk-2 half-x slab planes (px = 0..18 even)
NZS = 19         # z-padded: zi_slab = zi_global + 2, covering zofs in [-2, 1]
OXC = 8          # out x-planes per core
PAIRS = [(0, 0), (0, 1), (0, 2), (1, 1), (1, 2), (2, 2)]
VAR_S_DIV = 1.0 / (B * 16 * 16 * 16)
VAR_V_DIV = 1.0 / (B * 3 * 16 * 16 * 16)

SLAB_SHAPE = (128, NXS, 32, 2, NZS)    # [ci, px, iy, pz, zi]
SLAB2_SHAPE = (128, NXS2, 32, 2, NZS)  # [4x32 shifted c2, xi=px/2, iy, pz, zi]
WA_COLS = 448   # 7 single-tap blocks: pair cols [k0|k2][k1|k3][k4|k6][k5]
W2_COLS = 128   # [g0: kx=a | g1: kx=4+a] for row block a

KX_PAIRS = [(0, 2), (1, 3), (4, 6)]
WC_TAP = {0: 0, 2: 64, 1: 128, 3: 192, 4: 256, 6: 320, 5: 384}


# ---------------------------------------------------------------- host prep

def _assemble_kernel_sym(inp):
    """Assemble the dense conv kernel [64, 208, 7,7,7] and symmetrize the
    t-block -> [64, 160, 7,7,7]."""
    def blk(w, basis):
        w = np.asarray(w, np.float32)
        basis = np.asarray(basis, np.float32)
        mo, mi, nb = w.shape
        do, di = basis.shape[1], basis.shape[2]
        k = np.einsum('uvb,bijxyz->uivjxyz', w, basis)
        return k.reshape(mo * do, mi * di, SIZE, SIZE, SIZE)

    row_s = np.concatenate([blk(inp['w_ss'], inp['basis_ss']),
                            blk(inp['w_sv'], inp['basis_sv']),
                            blk(inp['w_st'], inp['basis_st'])], axis=1)
    row_v = np.concatenate([blk(inp['w_vs'], inp['basis_vs']),
                            blk(inp['w_vv'], inp['basis_vv']),
                            blk(inp['w_vt'], inp['basis_vt'])], axis=1)
    K = np.concatenate([row_s, row_v], axis=0)  # [64, 208, 7,7,7]

    Ks = np.empty((CO, CI, SIZE, SIZE, SIZE), np.float32)
    Ks[:, :64] = K[:, :64]
    for u in range(16):
        for pi, (i, j) in enumerate(PAIRS):
            src = K[:, 64 + 9 * u + 3 * i + j]
            if i != j:
                src = src + K[:, 64 + 9 * u + 3 * j + i]
            Ks[:, 64 + 6 * u + pi] = src
    return Ks


def _svt_sym(sv):
    """[4,64,32,32,32] -> symmetrized tensor-product features [4,160,32,32,32]."""
    sv = np.asarray(sv, np.float32)
    s = sv[:, :S_IN]
    v = sv[:, S_IN:].reshape(B, V_IN, 3, 32, 32, 32)
    t = np.empty((B, V_IN, 6, 32, 32, 32), np.float32)
    for pi, (i, j) in enumerate(PAIRS):
        t[:, :, pi] = v[:, :, i] * v[:, :, j]
    return np.concatenate([s, v.reshape(B, 48, 32, 32, 32),
                           t.reshape(B, 96, 32, 32, 32)], axis=1)


def _core_slabs(svt, b, h):
    """x/z zero-padded, z-parity-split slabs for core (b, h).

    Returns (c1, c2e): c1 SLAB_SHAPE, plane px holds global ix = px + 16h - 3;
    c2e SLAB2_SHAPE, block a (rows 32a:32a+32) of plane xi holds chunk-2
    channels at ix = 2*xi + a + 16h - 3. zi_slab = zi_global + 2.
    """
    sp = svt[b].reshape(CI, 32, 32, 16, 2)   # (ci, x, y, zi, pz); iz = 2*zi + pz
    sp = np.moveaxis(sp, 4, 3)               # (ci, x, y, pz, zi)
    x0 = 16 * h - 3
    c1 = np.zeros(SLAB_SHAPE, np.float32)
    lo, hi = max(0, x0), min(32, x0 + NXS)
    c1[:, lo - x0:hi - x0, :, :, 2:18] = sp[:128, lo:hi]
    c2 = np.zeros(SLAB2_SHAPE, np.float32)
    for a in range(4):
        for xi in range(NXS2):
            ix = 2 * xi + a + x0
            if 0 <= ix < 32:
                c2[32 * a:32 * a + 32, xi, :, :, 2:18] = sp[128:160, ix]
    return c1, c2


def _weight_slabs(Ks):
    """(WA, W2): chunk-1 tap blocks [49, 128, 448] and the 4-way kx-merged
    chunk-2 slab [49, 128, 128]."""
    WA = np.zeros((49, 128, WA_COLS), np.float32)
    W2 = np.zeros((49, 128, W2_COLS), np.float32)
    for ky, kz in product(range(SIZE), range(SIZE)):
        i = ky * SIZE + kz
        for kx, off in WC_TAP.items():
            WA[i, :, off:off + 64] = Ks[:, :128, kx, ky, kz].T
        for g in range(2):
            for a in range(4):
                kx = 4 * g + a
                if kx > 6:
                    continue
                W2[i, 32 * a:32 * a + 32, 64 * g:64 * (g + 1)] = \
                    Ks[:, 128:160, kx, ky, kz].T
    return WA, W2


def _gam_bias(bn_g_s, bn_g_v, bias_s):
    """Per-channel gamma [64] (vector gammas replicated x3) and bias [64]."""
    gam = np.empty(64, np.float32)
    gam[:16] = np.asarray(bn_g_s, np.float32)
    gam[16:] = np.repeat(np.asarray(bn_g_v, np.float32), 3)
    bias = np.zeros(64, np.float32)
    bias[:16] = np.asarray(bias_s, np.float32)
    return gam, bias


def _aux_mat():
    """[128, 128] constant: cols 0:64 on rows 0:64 = variance-combining
    matrix C (vloc = C.T @ ssq); cols 64:128 on rows 64:128 = identity (the
    partition-shift matmul that moves upper-half psum partials down)."""
    aux = np.zeros((128, 128), np.float32)
    for c in range(16):
        aux[c, c] = VAR_S_DIV
    for c in range(16, 64):
        g = (c - 16) // 3
        for j in range(3):
            aux[16 + 3 * g + j, c] = VAR_V_DIV
    aux[64:, 64:] = np.eye(64, dtype=np.float32)
    return aux


# ---------------------------------------------------------------- matmul plan

def _box(ky, kz):
    """Valid output range + slab coords for kernel offsets (ky, kz)."""
    d = kz - 3
    p = d % 2
    zofs = (d - p) // 2
    oy0 = max(0, (4 - ky) // 2)
    oy1 = min(16, (34 - ky) // 2 + 1)
    iy0 = 2 * oy0 + ky - 3
    return dict(p=p, zs=zofs + 2, iy0=iy0, oyc=oy1 - oy0, oy0=oy0)


def _mm_plan():
    """Matmul descriptors in issue order.

    Each entry: (i, slab, xi0, nx, wc, ww, bank, h0, nh, p0, p1) where
    slab 2 reads sl2[:, xi0:xi0+nx] (stride 1), slab 1 reads
    sl1[:, xi0:xi0+2*nx:2]; lhsT = W{a,2}[i][:, wc:wc+ww]; out =
    pq[bank][p0:p1, h0:h0+nh, oy...]. Chunk-2 phase first (its psum banks
    finish early and its compute covers the chunk-1 slab's DMA).
    """
    plan = []
    for i in range(49):
        plan.append((i, 2, 0, 2, 448 + 0, 64, 0, 0, 2, 0, 64))    # c2s g0 -> planes 0,1 L
        for q in range(3):                                         # c2 full
            plan.append((i, 2, 2 + 2 * q, 2, 448, 128, 5 + q, 0, 2, 0, 128))
        plan.append((i, 2, 8, 2, 448 + 64, 64, 3, 0, 2, 0, 64))    # c2s g1 -> planes 6,7 L
    for i in range(49):
        for ka, kb in KX_PAIRS:
            wc = WC_TAP[ka]
            assert WC_TAP[kb] == wc + 64
            for m in range(4):
                plan.append((i, 1, ka + 4 * m, 2, wc, 128, m, 0, 2, 0, 128))
            # edge: plane 7 via tap kb at px = ka+16 -> slot 8 LOWER (psum
            # writes at partition offset 64 fail the fp32r ISA check)
            plan.append((i, 1, ka + 16, 1, wc + 64, 64, 4, 0, 1, 0, 64))
        for m in range(4):                                         # single tap 5
            plan.append((i, 1, 5 + 4 * m, 2, WC_TAP[5], 64, m, 0, 2, 0, 64))
    return plan


def _regions(d):
    """(slot, 'L'/'U') psum regions written by descriptor d."""
    _, _, _, _, _, _, bank, h0, nh, p0, p1 = d
    out = []
    for dh in range(nh):
        s = 2 * bank + h0 + dh
        if p0 == 0:
            out.append((s, 'L'))
        if p1 == 128:
            out.append((s, 'U'))
    return out


_PLAN = _mm_plan()
_LAST_IDX = {}
for _n, _d in enumerate(_PLAN):
    for _r in _regions(_d):
        _LAST_IDX[_r] = _n
_STOPS = set(_LAST_IDX.values())
N_C2 = 49 * 5


# ---------------------------------------------------------------- numpy shadow

def _shadow_core(c1, c2, WA, W2):
    """Execute the matmul plan in numpy. Returns conv output [64, 8, 16, 16]."""
    # psum: [bank, half, part, y, z]
    ps = np.zeros((8, 2, 128, 16, 16), np.float32)
    for i, slab, xi0, nx, wc, ww, bank, h0, nh, p0, p1 in _PLAN:
        ky, kz = divmod(i, 7)
        bx = _box(ky, kz)
        W = W2 if wc >= 448 else WA
        lhsT = W[i][:, (wc - 448 if wc >= 448 else wc):][:, :ww]
        sl = c2 if slab == 2 else c1
        step = 1 if slab == 2 else 2
        for dh in range(nh):
            xi = xi0 + step * dh
            rhs = sl[:, xi, bx['iy0']:bx['iy0'] + 2 * bx['oyc']:2, bx['p'],
                     bx['zs']:bx['zs'] + 16]
            contrib = np.einsum('km,kbc->mbc', lhsT, rhs)
            ys = slice(bx['oy0'], bx['oy0'] + bx['oyc'])
            ps[bank, h0 + dh, p0:p1, ys, :] += contrib
    # slot s = (bank s//2, half s%2); L = parts 0:64, U = 64:128
    def L(s):
        return ps[s // 2, s % 2, 0:64]

    def U(s):
        return ps[s // 2, s % 2, 64:128]

    out = np.empty((OXC, CO, 16, 16), np.float32)
    for j in range(OXC):
        out[j] = L(j) + (U(j + 1) if j <= 6 else L(8))
        if j >= 2:
            out[j] += L(10 + (j - 2))
        if j <= 5:
            out[j] += U(10 + j)
    return out.transpose(1, 0, 2, 3)


def shadow_forward(inp):
    """Full-model numpy shadow of the device computation (for plan validation)."""
    svt = _svt_sym(inp['sv'])
    Ks = _assemble_kernel_sym(inp)
    WA, W2 = _weight_slabs(Ks)
    gam, bias = _gam_bias(inp['bn_g_s'], inp['bn_g_v'], inp['bias_s'])

    y = np.zeros((B, CO, 16, 16, 16), np.float32)
    ss = np.zeros(64, np.float64)
    for c in range(NCORES):
        b, h = c // 2, c % 2
        c1, c2 = _core_slabs(svt, b, h)
        out = _shadow_core(c1, c2, WA, W2)
        y[b, :, 8 * h:8 * h + 8] = out
        ss += (out.astype(np.float64) ** 2).sum(axis=(1, 2, 3))

    var = np.empty(64)
    var[:16] = ss[:16] * VAR_S_DIV
    vv = (ss[16::3] + ss[17::3] + ss[18::3]) * VAR_V_DIV
    var[16:] = np.repeat(vv, 3)
    scale = gam / np.sqrt(var + EPS)
    y = y * scale[None, :, None, None, None].astype(np.float32)
    y[:, :16] = np.maximum(y[:, :16] + bias[:16][None, :, None, None, None], 0.0)
    return y


# ---------------------------------------------------------------- bass kernel

_CACHED = {}


def _build_bass():
    import concourse.bass as bass
    import concourse.tile as tile
    import concourse.mybir as mybir
    from concourse import bacc

    f32 = mybir.dt.float32
    f32r = mybir.dt.float32r

    nc = bacc.Bacc("TRN2", target_bir_lowering=False, debug=False, num_devices=NCORES)

    in1 = nc.dram_tensor("in1", list(SLAB_SHAPE), f32r, kind="ExternalInput").ap()
    in2 = nc.dram_tensor("in2", list(SLAB2_SHAPE), f32r, kind="ExternalInput").ap()
    wa_in = nc.dram_tensor("wa_in", [49, 128, WA_COLS], f32r, kind="ExternalInput").ap()
    w2_in = nc.dram_tensor("w2_in", [128, 49 * W2_COLS], f32r, kind="ExternalInput").ap()
    aux_in = nc.dram_tensor("aux_in", [128, 128], f32, kind="ExternalInput").ap()
    gam_in = nc.dram_tensor("gam_in", [64, 1], f32, kind="ExternalInput").ap()
    bias_in = nc.dram_tensor("bias_in", [64, 1], f32, kind="ExternalInput").ap()
    out_d = nc.dram_tensor("out", [CO, OXC, 16, 16], f32, kind="ExternalOutput").ap()

    with tile.TileContext(nc) as tc:
        with (
            tc.tile_pool(name="slab", bufs=1) as slab_pool,
            tc.tile_pool(name="wpa", bufs=6) as wpa_pool,
            tc.tile_pool(name="ps", bufs=1, space="PSUM") as ps,
            tc.tile_pool(name="outp", bufs=1) as outp,
            tc.tile_pool(name="stat", bufs=1) as stat,
            tc.tile_pool(name="dram", bufs=1, space="DRAM") as dram,
        ):
            # 8 psum banks = 16 half-bank slots
            pq = [ps.tile([128, 2, 16, 16], f32, tag=f"pq{t}", name=f"pq{t}")
                  for t in range(8)]

            # inputs balanced across the two HWDGE queues: scalar gets sl2
            # (gates the chunk-2 phase plane-by-plane) then the sl1 tail;
            # sync gets the sl1 head. Each queue moves ~10MB (~45us).
            sl1 = slab_pool.tile(list(SLAB_SHAPE), f32r, tag="slab", name="slab_c1")
            sl2 = slab_pool.tile(list(SLAB2_SHAPE), f32r, tag="slab2",
                                 name="slab_c2")
            for xi in range(NXS2):
                nc.scalar.dma_start(sl2[:, xi], in2[:, xi])

            # HBM priority order: the chunk-2 phase's inputs (w2, sl2) go
            # FIRST in the shared HBM pipe so the tensor engine isn't idle
            # during the big sl1/wa load. w2 lives in one [128, 49*128] tile
            # (partition-major DRAM layout) loaded in 4 chunked DMAs.
            aux_t = stat.tile([128, 128], f32, tag="aux")
            w2s = slab_pool.tile([128, 49 * W2_COLS], f32r, tag="w2s",
                                 name="w2s")
            wat = [wpa_pool.tile([128, WA_COLS], f32r, tag="wa", name=f"wa_{i}")
                   for i in range(49)]
            for c0, c1 in ((0, 1664), (1664, 3328), (3328, 4992), (4992, 6272)):
                nc.sync.dma_start(w2s[:, c0:c1], w2_in[:, c0:c1])
            nc.sync.dma_start(aux_t[:], aux_in[:])
            for px in range(14):
                nc.sync.dma_start(sl1[:, px], in1[:, px])
            for px in range(14, NXS):
                nc.scalar.dma_start(sl1[:, px], in1[:, px])
            for i in range(49):
                nc.sync.dma_start(wat[i][:], wa_in[i])

            # start=True clears the WHOLE psum bank, so open each bank once
            # with a zero-weight full-bank matmul (also a WAW dep that orders
            # it before every accumulate); all real matmuls use start=False.
            # The rhs is our own memset tile so openers don't wait on input
            # DMAs.
            zw_f = stat.tile([128, 512], f32, tag="zw")
            nc.vector.memset(zw_f[:], 0.0)
            zw = zw_f.bitcast(f32r)
            for t in range(8):
                nc.tensor.matmul(pq[t].rearrange("c a y z -> c (a y z)"),
                                 zw[:, 0:128], zw[:, :], start=True, stop=False)

            # warm the sqrt/square/relu activation table on the scalar engine
            # now so the tail's activations don't pay the 1.3us table load
            eps_t = stat.tile([CO, 1], f32, tag="eps")
            nc.vector.memset(eps_t[:], EPS)
            sd = stat.tile([CO, 1], f32, tag="sd")
            nc.scalar.activation(sd[0:1], eps_t[0:1],
                                 mybir.ActivationFunctionType.Sqrt)

            def emit(n, d):
                i, slab, xi0, nx, wc, ww, bank, h0, nh, p0, p1 = d
                ky, kz = divmod(i, 7)
                bx = _box(ky, kz)
                if wc >= 448:
                    w = w2s
                    wc = W2_COLS * i + (wc - 448)
                else:
                    w = wat[i]
                if slab == 2:
                    rhs = sl2[:, xi0:xi0 + nx, bx['iy0']:bx['iy0'] + 2 * bx['oyc'] - 1:2,
                              bx['p'], bx['zs']:bx['zs'] + 16]
                else:
                    rhs = sl1[:, xi0:xi0 + 2 * nx - 1:2,
                              bx['iy0']:bx['iy0'] + 2 * bx['oyc'] - 1:2,
                              bx['p'], bx['zs']:bx['zs'] + 16]
                out_ap = pq[bank][p0:p1, h0:h0 + nh, bx['oy0']:bx['oy0'] + bx['oyc'], :]
                nc.tensor.matmul(out_ap, w[:, wc:wc + ww], rhs,
                                 start=False, stop=n in _STOPS)

            for n in range(N_C2):
                emit(n, _PLAN[n])

            # early evacuation of chunk2-only banks 5-7, overlapped with the
            # chunk-1 matmuls: LB -> osb planes 2..7, UB -> usb planes 0..5
            osb = outp.tile([CO, OXC, 16, 16], f32, tag="osb")
            usb = outp.tile([128, OXC, 16, 16], f32, tag="usb")
            for q in range(3):
                nc.vector.tensor_copy(osb[:, 2 + 2 * q:4 + 2 * q], pq[5 + q][0:64])
                nc.vector.tensor_copy(usb[64:128, 2 * q:2 * q + 2], pq[5 + q][64:128])

            for n in range(N_C2, len(_PLAN)):
                emit(n, _PLAN[n])

            # late evacuation: plane j needs L(j) [slots 0..7], U(j+1) for
            # j<=6 [slots 1..7], and plane 7 the edge partials from slot 8 L.
            # usb (U halves) first: the shift matmuls depend only on it, so
            # they overlap the osb ops that follow.
            nc.vector.tensor_add(usb[64:128, 0], usb[64:128, 0], pq[0][64:128, 1])
            nc.vector.tensor_add(usb[64:128, 1:3], usb[64:128, 1:3], pq[1][64:128])
            nc.vector.tensor_add(usb[64:128, 3:5], usb[64:128, 3:5], pq[2][64:128])
            nc.vector.tensor_add(usb[64:128, 5], usb[64:128, 5], pq[3][64:128, 0])
            nc.vector.tensor_copy(usb[64:128, 6], pq[3][64:128, 1])
            nc.vector.tensor_copy(osb[:, 0:2], pq[0][0:64])
            for m in range(1, 4):
                nc.vector.tensor_add(osb[:, 2 * m:2 * m + 2], osb[:, 2 * m:2 * m + 2],
                                     pq[m][0:64])
            nc.vector.tensor_add(osb[:, 7], osb[:, 7], pq[4][0:64, 0])

            # move upper-half partials (planes 0..6) down to partitions 0:64
            # via identity matmuls through the (now free) psum banks 0-3 —
            # no DRAM round-trip — then add into osb
            ident = aux_t[64:128, 64:128]
            for m in range(3):
                nc.tensor.matmul(pq[m][0:64], ident,
                                 usb[64:128, 2 * m:2 * m + 2],
                                 start=True, stop=True)
            nc.tensor.matmul(pq[3][0:64, 0], ident, usb[64:128, 6],
                             start=True, stop=True)
            for m in range(3):
                nc.vector.tensor_add(osb[:, 2 * m:2 * m + 2],
                                     osb[:, 2 * m:2 * m + 2], pq[m][0:64])
            nc.vector.tensor_add(osb[:, 6], osb[:, 6], pq[3][0:64, 0])

            # per-channel sum of squares in one fused scalar-engine op
            # (square + per-partition accumulate), then one matmul with the
            # combining matrix C turns it into the local variance [64, 1]
            of = osb.rearrange("c x y z -> c (x y z)")
            sq = usb.rearrange("c x y z -> c (x y z)")[0:64, :]
            ssq = stat.tile([CO, 1], f32, tag="ssq")
            ssq_b = stat.tile([CO, 1], f32, tag="ssqb")
            nc.scalar.activation(sq[:, 0:1024], of[:, 0:1024],
                                 mybir.ActivationFunctionType.Square,
                                 accum_out=ssq[:])
            nc.vector.tensor_mul(sq[:, 1024:2048], of[:, 1024:2048],
                                 of[:, 1024:2048])
            nc.vector.tensor_reduce(ssq_b[:], sq[:, 1024:2048],
                                    axis=mybir.AxisListType.X,
                                    op=mybir.AluOpType.add)
            nc.vector.tensor_add(ssq[:], ssq[:], ssq_b[:])
            nc.tensor.matmul(pq[5][0:64, 0, 0, 0:1], aux_t[0:64, 0:64],
                             ssq[:, :], start=True, stop=True)
            vcol = stat.tile([CO, 1], f32, tag="vcol")
            nc.vector.tensor_copy(vcol[:], pq[5][0:64, 0, 0, 0:1])

            v_dram = dram.tile([1, 64], f32, tag="vd")
            v_red = dram.tile([1, 64], f32, tag="vr")
            nc.sync.dma_start(v_dram[0, :], vcol[:, 0])
            nc.gpsimd.collective_compute(
                "AllReduce", mybir.AluOpType.add,
                replica_groups=[list(range(NCORES))],
                ins=[v_dram.opt()], outs=[v_red.opt()],
            )

            # scale = gamma / sqrt(var + eps), in per-partition layout
            var_col = stat.tile([CO, 1], f32, tag="varcol")
            nc.sync.dma_start(var_col[:, 0], v_red[0, :])
            nc.scalar.activation(sd[:], var_col[:], mybir.ActivationFunctionType.Sqrt,
                                 bias=eps_t[:], scale=1.0)
            inv = stat.tile([CO, 1], f32, tag="inv")
            nc.vector.reciprocal(inv[:], sd[:])
            gam_t = stat.tile([CO, 1], f32, tag="gam")
            nc.sync.dma_start(gam_t[:], gam_in[:])
            scale_col = stat.tile([CO, 1], f32, tag="sccol")
            nc.vector.tensor_mul(scale_col[:], inv[:], gam_t[:])
            bias_t = stat.tile([CO, 1], f32, tag="bias")
            nc.sync.dma_start(bias_t[:], bias_in[:])

            # BN scale everywhere (DVE partition ranges must be 32-aligned,
            # so no scalar/vector split), then bias+relu on scalar channels;
            # the vector-channel store only waits on the scale
            nc.vector.tensor_scalar_mul(of[:, :], of[:, :], scale_col[:, :])
            nc.scalar.activation(of[0:16, :], of[0:16, :],
                                 mybir.ActivationFunctionType.Relu,
                                 bias=bias_t[0:16, :], scale=1.0)
            # vector-channel store needs only the scale; it rides sync so it
            # overlaps the scalar-engine relu, whose store follows on scalar
            nc.sync.dma_start(out_d[16:64], osb[16:64])
            nc.scalar.dma_start(out_d[0:16], osb[0:16])

    nc.compile()
    return nc


def _install_ntff_hook():
    import sys, types
    if "antenv.axon_hooks" in sys.modules:
        return
    mod = types.ModuleType("antenv.axon_hooks")
    mod._hook = None
    mod.set_axon_ntff_profile_hook = lambda h: setattr(mod, "_hook", h)
    mod.get_axon_ntff_profile_hook = lambda: mod._hook
    sys.modules["antenv.axon_hooks"] = mod
    try:
        import antenv
        antenv.axon_hooks = mod
        from trn_agent_boot.trn_boot import _ntff_profile_via_ctypes
        mod.set_axon_ntff_profile_hook(_ntff_profile_via_ctypes("/opt/axon/libaxon_pjrt.so"))
    except Exception:
        pass


def run_on_hw(inp, trace=False):
    """Run the kernel on 8 cores. Returns (full output [4,64,16,16,16], results)."""
    from concourse.bass_utils import run_bass_kernel_spmd

    if "nc" not in _CACHED:
        _install_ntff_hook()
        _CACHED["nc"] = _build_bass()
    nc = _CACHED["nc"]

    svt = _svt_sym(inp['sv'])
    Ks = _assemble_kernel_sym(inp)
    WA, W2 = _weight_slabs(Ks)
    gam, bias = _gam_bias(inp['bn_g_s'], inp['bn_g_v'], inp['bias_s'])

    in_maps = []
    for c in range(NCORES):
        b, h = c // 2, c % 2
        c1, c2 = _core_slabs(svt, b, h)
        in_maps.append({
            "in1": c1,
            "in2": c2,
            "wa_in": WA,
            "w2_in": np.ascontiguousarray(W2.transpose(1, 0, 2).reshape(128, 49 * W2_COLS)),
            "aux_in": _aux_mat(),
            "gam_in": gam.reshape(64, 1),
            "bias_in": bias.reshape(64, 1),
        })

    res = run_bass_kernel_spmd(nc, in_maps, core_ids=list(range(NCORES)), trace=trace)

    y = np.zeros((B, CO, 16, 16, 16), np.float32)
    for c in range(NCORES):
        b, h = c // 2, c % 2
        y[b, :, 8 * h:8 * h + 8] = res.results[c]["out"]
    return y, res


def kernel(**inputs) -> np.ndarray:
    y, _ = run_on_hw(inputs, trace=False)
    return y


# revision 23
# speedup vs baseline: 1.1723x; 1.1018x over previous
"""SE(3)-CNN block (TensorProduct -> SE3Conv -> SE3BatchNorm -> BiasRelu) on 8 trn2 cores.

Sharding: core c = (batch b=c//2, out-x-half h=c%2). Each core computes all 64
output channels for 8 of 16 output x-planes of one batch; per-field BN second
moments are combined with a tiny [1,64] AllReduce across all 8 cores.

Conv strategy: the 9 t-channels per vector pair are symmetric (t = v (x) v), so
the 208 input channels reduce to 160 symmetrized ones. The contraction runs as
fp32r matmuls, one per (ky, kz, kx-pair, psum-bank), with free dim spanning TWO
output-x planes (2 x oyc x 16 <= 512 = one full psum bank) so each instruction
streams ~484 elements. kx tap pairs (ka, ka+2) share one rhs read: lhsT cols
0:64 = tap ka (accumulates plane (px-ka)/2), cols 64:128 = tap ka+2 (plane one
lower), using the slot trick: psum slot s holds plane s in partitions 0:64 and
plane s-1 in partitions 64:128. Slot s = (bank s//2, half s%2); a double-slot
matmul covers slots (2m, 2m+1) = bank m. Chunk1 (channels 0:128) uses slots
0..8 (banks 0-4); chunk2 (channels 128:160, stored as 4 x-shifted copies so 4
kx taps pack into 128 contraction rows) uses slots 10..15 (banks 5-7) plus
edge writes into chunk1 slots. Chunk2 runs first so its psum banks finish ~60us
in and evacuate overlapped with chunk1 matmuls.
"""
import numpy as np
from itertools import product

# problem constants (from spec / reference)
B = 4
S_IN = 16
V_IN = 16
CO = 64          # 16 scalar + 48 vector output channels
CI = 160         # 16 s + 48 v + 96 t_sym
SIZE = 7
PAD = 3
STRIDE = 2
EPS = 1e-5
NCORES = 8
NXS = 21         # x-padded slab planes per core (px 0..20 read)
NXS2 = 10        # chunk-2 half-x slab planes (px = 0..18 even)
NZS = 19         # z-padded: zi_slab = zi_global + 2, covering zofs in [-2, 1]
OXC = 8          # out x-planes per core
PAIRS = [(0, 0), (0, 1), (0, 2), (1, 1), (1, 2), (2, 2)]
VAR_S_DIV = 1.0 / (B * 16 * 16 * 16)
VAR_V_DIV = 1.0 / (B * 3 * 16 * 16 * 16)

SLAB_SHAPE = (128, NXS, 32, 2, NZS)    # [ci, px, iy, pz, zi]
SLAB2_SHAPE = (128, NXS2, 32, 2, NZS)  # [4x32 shifted c2, xi=px/2, iy, pz, zi]
WA_COLS = 448   # 7 single-tap blocks: pair cols [k0|k2][k1|k3][k4|k6][k5]
W2_COLS = 128   # [g0: kx=a | g1: kx=4+a] for row block a

KX_PAIRS = [(0, 2), (1, 3), (4, 6)]
WC_TAP = {0: 0, 2: 64, 1: 128, 3: 192, 4: 256, 6: 320, 5: 384}


# ---------------------------------------------------------------- host prep

def _assemble_kernel_sym(inp):
    """Assemble the dense conv kernel [64, 208, 7,7,7] and symmetrize the
    t-block -> [64, 160, 7,7,7]."""
    def blk(w, basis):
        w = np.asarray(w, np.float32)
        basis = np.asarray(basis, np.float32)
        mo, mi, nb = w.shape
        do, di = basis.shape[1], basis.shape[2]
        k = np.einsum('uvb,bijxyz->uivjxyz', w, basis)
        return k.reshape(mo * do, mi * di, SIZE, SIZE, SIZE)

    row_s = np.concatenate([blk(inp['w_ss'], inp['basis_ss']),
                            blk(inp['w_sv'], inp['basis_sv']),
                            blk(inp['w_st'], inp['basis_st'])], axis=1)
    row_v = np.concatenate([blk(inp['w_vs'], inp['basis_vs']),
                            blk(inp['w_vv'], inp['basis_vv']),
                            blk(inp['w_vt'], inp['basis_vt'])], axis=1)
    K = np.concatenate([row_s, row_v], axis=0)  # [64, 208, 7,7,7]

    Ks = np.empty((CO, CI, SIZE, SIZE, SIZE), np.float32)
    Ks[:, :64] = K[:, :64]
    for u in range(16):
        for pi, (i, j) in enumerate(PAIRS):
            src = K[:, 64 + 9 * u + 3 * i + j]
            if i != j:
                src = src + K[:, 64 + 9 * u + 3 * j + i]
            Ks[:, 64 + 6 * u + pi] = src
    return Ks


def _svt_sym(sv):
    """[4,64,32,32,32] -> symmetrized tensor-product features [4,160,32,32,32]."""
    sv = np.asarray(sv, np.float32)
    s = sv[:, :S_IN]
    v = sv[:, S_IN:].reshape(B, V_IN, 3, 32, 32, 32)
    t = np.empty((B, V_IN, 6, 32, 32, 32), np.float32)
    for pi, (i, j) in enumerate(PAIRS):
        t[:, :, pi] = v[:, :, i] * v[:, :, j]
    return np.concatenate([s, v.reshape(B, 48, 32, 32, 32),
                           t.reshape(B, 96, 32, 32, 32)], axis=1)


def _core_slabs(svt, b, h):
    """x/z zero-padded, z-parity-split slabs for core (b, h).

    Returns (c1, c2e): c1 SLAB_SHAPE, plane px holds global ix = px + 16h - 3;
    c2e SLAB2_SHAPE, block a (rows 32a:32a+32) of plane xi holds chunk-2
    channels at ix = 2*xi + a + 16h - 3. zi_slab = zi_global + 2.
    """
    sp = svt[b].reshape(CI, 32, 32, 16, 2)   # (ci, x, y, zi, pz); iz = 2*zi + pz
    sp = np.moveaxis(sp, 4, 3)               # (ci, x, y, pz, zi)
    x0 = 16 * h - 3
    c1 = np.zeros(SLAB_SHAPE, np.float32)
    lo, hi = max(0, x0), min(32, x0 + NXS)
    c1[:, lo - x0:hi - x0, :, :, 2:18] = sp[:128, lo:hi]
    c2 = np.zeros(SLAB2_SHAPE, np.float32)
    for a in range(4):
        for xi in range(NXS2):
            ix = 2 * xi + a + x0
            if 0 <= ix < 32:
                c2[32 * a:32 * a + 32, xi, :, :, 2:18] = sp[128:160, ix]
    return c1, c2


def _weight_slabs(Ks):
    """(WA, W2): chunk-1 tap blocks [49, 128, 448] and the 4-way kx-merged
    chunk-2 slab [49, 128, 128]."""
    WA = np.zeros((49, 128, WA_COLS), np.float32)
    W2 = np.zeros((49, 128, W2_COLS), np.float32)
    for ky, kz in product(range(SIZE), range(SIZE)):
        i = ky * SIZE + kz
        for kx, off in WC_TAP.items():
            WA[i, :, off:off + 64] = Ks[:, :128, kx, ky, kz].T
        for g in range(2):
            for a in range(4):
                kx = 4 * g + a
                if kx > 6:
                    continue
                W2[i, 32 * a:32 * a + 32, 64 * g:64 * (g + 1)] = \
                    Ks[:, 128:160, kx, ky, kz].T
    return WA, W2


def _gam_bias(bn_g_s, bn_g_v, bias_s):
    """Per-channel gamma [64] (vector gammas replicated x3) and bias [64]."""
    gam = np.empty(64, np.float32)
    gam[:16] = np.asarray(bn_g_s, np.float32)
    gam[16:] = np.repeat(np.asarray(bn_g_v, np.float32), 3)
    bias = np.zeros(64, np.float32)
    bias[:16] = np.asarray(bias_s, np.float32)
    return gam, bias


def _aux_mat():
    """[128, 128] constant: cols 0:64 on rows 0:64 = variance-combining
    matrix C (vloc = C.T @ ssq); cols 64:128 on rows 64:128 = identity (the
    partition-shift matmul that moves upper-half psum partials down)."""
    aux = np.zeros((128, 128), np.float32)
    for c in range(16):
        aux[c, c] = VAR_S_DIV
    for c in range(16, 64):
        g = (c - 16) // 3
        for j in range(3):
            aux[16 + 3 * g + j, c] = VAR_V_DIV
    aux[64:, 64:] = np.eye(64, dtype=np.float32)
    return aux


# ---------------------------------------------------------------- matmul plan

def _box(ky, kz):
    """Valid output range + slab coords for kernel offsets (ky, kz)."""
    d = kz - 3
    p = d % 2
    zofs = (d - p) // 2
    oy0 = max(0, (4 - ky) // 2)
    oy1 = min(16, (34 - ky) // 2 + 1)
    iy0 = 2 * oy0 + ky - 3
    return dict(p=p, zs=zofs + 2, iy0=iy0, oyc=oy1 - oy0, oy0=oy0)


def _mm_plan():
    """Matmul descriptors in issue order.

    Each entry: (i, slab, xi0, nx, wc, ww, bank, h0, nh, p0, p1) where
    slab 2 reads sl2[:, xi0:xi0+nx] (stride 1), slab 1 reads
    sl1[:, xi0:xi0+2*nx:2]; lhsT = W{a,2}[i][:, wc:wc+ww]; out =
    pq[bank][p0:p1, h0:h0+nh, oy...]. Chunk-2 phase first (its psum banks
    finish early and its compute covers the chunk-1 slab's DMA).
    """
    plan = []
    for i in range(49):
        plan.append((i, 2, 0, 2, 448 + 0, 64, 0, 0, 2, 0, 64))    # c2s g0 -> planes 0,1 L
        for q in range(3):                                         # c2 full
            plan.append((i, 2, 2 + 2 * q, 2, 448, 128, 5 + q, 0, 2, 0, 128))
        plan.append((i, 2, 8, 2, 448 + 64, 64, 3, 0, 2, 0, 64))    # c2s g1 -> planes 6,7 L
    def tap_mms(i):
        out = []
        for ka, kb in KX_PAIRS:
            wc = WC_TAP[ka]
            assert WC_TAP[kb] == wc + 64
            for m in range(4):
                out.append((i, 1, ka + 4 * m, 2, wc, 128, m, 0, 2, 0, 128))
            # edge: plane 7 via tap kb at px = ka+16 -> slot 8 LOWER (psum
            # writes at partition offset 64 fail the fp32r ISA check)
            out.append((i, 1, ka + 16, 1, wc + 64, 64, 4, 0, 1, 0, 64))
        for m in range(4):                                         # single tap 5
            out.append((i, 1, 5 + 4 * m, 2, WC_TAP[5], 64, m, 0, 2, 0, 64))
        return out

    for i in range(43):
        plan.extend(tap_mms(i))
    # last 6 taps bank-major (bank 3 last): banks complete staggered so the
    # vector engine evacuates them during the final matmuls instead of in
    # one serialized burst after. 6 taps = exactly the wa tile-pool depth.
    tail = [d for i in range(43, 49) for d in tap_mms(i)]
    for bank in (0, 1, 2, 4, 3):
        plan.extend(d for d in tail if d[6] == bank)
    return plan


def _regions(d):
    """(slot, 'L'/'U') psum regions written by descriptor d."""
    _, _, _, _, _, _, bank, h0, nh, p0, p1 = d
    out = []
    for dh in range(nh):
        s = 2 * bank + h0 + dh
        if p0 == 0:
            out.append((s, 'L'))
        if p1 == 128:
            out.append((s, 'U'))
    return out


_PLAN = _mm_plan()
_LAST_IDX = {}
for _n, _d in enumerate(_PLAN):
    for _r in _regions(_d):
        _LAST_IDX[_r] = _n
_STOPS = set(_LAST_IDX.values())
N_C2 = 49 * 5


# ---------------------------------------------------------------- numpy shadow

def _shadow_core(c1, c2, WA, W2):
    """Execute the matmul plan in numpy. Returns conv output [64, 8, 16, 16]."""
    # psum: [bank, half, part, y, z]
    ps = np.zeros((8, 2, 128, 16, 16), np.float32)
    for i, slab, xi0, nx, wc, ww, bank, h0, nh, p0, p1 in _PLAN:
        ky, kz = divmod(i, 7)
        bx = _box(ky, kz)
        W = W2 if wc >= 448 else WA
        lhsT = W[i][:, (wc - 448 if wc >= 448 else wc):][:, :ww]
        sl = c2 if slab == 2 else c1
        step = 1 if slab == 2 else 2
        for dh in range(nh):
            xi = xi0 + step * dh
            rhs = sl[:, xi, bx['iy0']:bx['iy0'] + 2 * bx['oyc']:2, bx['p'],
                     bx['zs']:bx['zs'] + 16]
            contrib = np.einsum('km,kbc->mbc', lhsT, rhs)
            ys = slice(bx['oy0'], bx['oy0'] + bx['oyc'])
            ps[bank, h0 + dh, p0:p1, ys, :] += contrib
    # slot s = (bank s//2, half s%2); L = parts 0:64, U = 64:128
    def L(s):
        return ps[s // 2, s % 2, 0:64]

    def U(s):
        return ps[s // 2, s % 2, 64:128]

    out = np.empty((OXC, CO, 16, 16), np.float32)
    for j in range(OXC):
        out[j] = L(j) + (U(j + 1) if j <= 6 else L(8))
        if j >= 2:
            out[j] += L(10 + (j - 2))
        if j <= 5:
            out[j] += U(10 + j)
    return out.transpose(1, 0, 2, 3)


def shadow_forward(inp):
    """Full-model numpy shadow of the device computation (for plan validation)."""
    svt = _svt_sym(inp['sv'])
    Ks = _assemble_kernel_sym(inp)
    WA, W2 = _weight_slabs(Ks)
    gam, bias = _gam_bias(inp['bn_g_s'], inp['bn_g_v'], inp['bias_s'])

    y = np.zeros((B, CO, 16, 16, 16), np.float32)
    ss = np.zeros(64, np.float64)
    for c in range(NCORES):
        b, h = c // 2, c % 2
        c1, c2 = _core_slabs(svt, b, h)
        out = _shadow_core(c1, c2, WA, W2)
        y[b, :, 8 * h:8 * h + 8] = out
        ss += (out.astype(np.float64) ** 2).sum(axis=(1, 2, 3))

    var = np.empty(64)
    var[:16] = ss[:16] * VAR_S_DIV
    vv = (ss[16::3] + ss[17::3] + ss[18::3]) * VAR_V_DIV
    var[16:] = np.repeat(vv, 3)
    scale = gam / np.sqrt(var + EPS)
    y = y * scale[None, :, None, None, None].astype(np.float32)
    y[:, :16] = np.maximum(y[:, :16] + bias[:16][None, :, None, None, None], 0.0)
    return y


# ---------------------------------------------------------------- bass kernel

_CACHED = {}


def _build_bass():
    import concourse.bass as bass
    import concourse.tile as tile
    import concourse.mybir as mybir
    from concourse import bacc

    f32 = mybir.dt.float32
    f32r = mybir.dt.float32r

    nc = bacc.Bacc("TRN2", target_bir_lowering=False, debug=False, num_devices=NCORES)

    in1 = nc.dram_tensor("in1", list(SLAB_SHAPE), f32r, kind="ExternalInput").ap()
    in2 = nc.dram_tensor("in2", list(SLAB2_SHAPE), f32r, kind="ExternalInput").ap()
    wa_in = nc.dram_tensor("wa_in", [49, 128, WA_COLS], f32r, kind="ExternalInput").ap()
    w2_in = nc.dram_tensor("w2_in", [128, 49 * W2_COLS], f32r, kind="ExternalInput").ap()
    aux_in = nc.dram_tensor("aux_in", [128, 128], f32, kind="ExternalInput").ap()
    gam_in = nc.dram_tensor("gam_in", [64, 1], f32, kind="ExternalInput").ap()
    bias_in = nc.dram_tensor("bias_in", [64, 1], f32, kind="ExternalInput").ap()
    out_d = nc.dram_tensor("out", [CO, OXC, 16, 16], f32, kind="ExternalOutput").ap()

    with tile.TileContext(nc) as tc:
        with (
            tc.tile_pool(name="slab", bufs=1) as slab_pool,
            tc.tile_pool(name="wpa", bufs=6) as wpa_pool,
            tc.tile_pool(name="ps", bufs=1, space="PSUM") as ps,
            tc.tile_pool(name="outp", bufs=1) as outp,
            tc.tile_pool(name="stat", bufs=1) as stat,
            tc.tile_pool(name="dram", bufs=1, space="DRAM") as dram,
        ):
            # 8 psum banks = 16 half-bank slots
            pq = [ps.tile([128, 2, 16, 16], f32, tag=f"pq{t}", name=f"pq{t}")
                  for t in range(8)]

            # inputs balanced across the two HWDGE queues: scalar gets sl2
            # (gates the chunk-2 phase plane-by-plane) then the sl1 tail;
            # sync gets the sl1 head. Each queue moves ~10MB (~45us).
            sl1 = slab_pool.tile(list(SLAB_SHAPE), f32r, tag="slab", name="slab_c1")
            sl2 = slab_pool.tile(list(SLAB2_SHAPE), f32r, tag="slab2",
                                 name="slab_c2")
            for xi in range(NXS2):
                nc.scalar.dma_start(sl2[:, xi], in2[:, xi])

            # HBM priority order: the chunk-2 phase's inputs (w2, sl2) go
            # FIRST in the shared HBM pipe so the tensor engine isn't idle
            # during the big sl1/wa load. w2 lives in one [128, 49*128] tile
            # (partition-major DRAM layout) loaded in 4 chunked DMAs.
            aux_t = stat.tile([128, 128], f32, tag="aux")
            w2s = slab_pool.tile([128, 49 * W2_COLS], f32r, tag="w2s",
                                 name="w2s")
            wat = [wpa_pool.tile([128, WA_COLS], f32r, tag="wa", name=f"wa_{i}")
                   for i in range(49)]
            for c0, c1 in ((0, 1664), (1664, 3328), (3328, 4992), (4992, 6272)):
                nc.sync.dma_start(w2s[:, c0:c1], w2_in[:, c0:c1])
            nc.sync.dma_start(aux_t[:], aux_in[:])
            for px in range(14):
                nc.sync.dma_start(sl1[:, px], in1[:, px])
            for px in range(14, NXS):
                nc.scalar.dma_start(sl1[:, px], in1[:, px])
            for i in range(49):
                nc.sync.dma_start(wat[i][:], wa_in[i])

            # start=True clears the WHOLE psum bank, so open each bank once
            # with a zero-weight full-bank matmul (also a WAW dep that orders
            # it before every accumulate); all real matmuls use start=False.
            # The rhs is our own memset tile so openers don't wait on input
            # DMAs.
            zw_f = stat.tile([128, 512], f32, tag="zw")
            nc.vector.memset(zw_f[:], 0.0)
            zw = zw_f.bitcast(f32r)
            for t in range(8):
                nc.tensor.matmul(pq[t].rearrange("c a y z -> c (a y z)"),
                                 zw[:, 0:128], zw[:, :], start=True, stop=False)

            # warm the sqrt/square/relu activation table on the scalar engine
            # now so the tail's activations don't pay the 1.3us table load
            eps_t = stat.tile([CO, 1], f32, tag="eps")
            nc.vector.memset(eps_t[:], EPS)
            sd = stat.tile([CO, 1], f32, tag="sd")
            nc.scalar.activation(sd[0:1], eps_t[0:1],
                                 mybir.ActivationFunctionType.Sqrt)

            def emit(n, d):
                i, slab, xi0, nx, wc, ww, bank, h0, nh, p0, p1 = d
                ky, kz = divmod(i, 7)
                bx = _box(ky, kz)
                if wc >= 448:
                    w = w2s
                    wc = W2_COLS * i + (wc - 448)
                else:
                    w = wat[i]
                if slab == 2:
                    rhs = sl2[:, xi0:xi0 + nx, bx['iy0']:bx['iy0'] + 2 * bx['oyc'] - 1:2,
                              bx['p'], bx['zs']:bx['zs'] + 16]
                else:
                    rhs = sl1[:, xi0:xi0 + 2 * nx - 1:2,
                              bx['iy0']:bx['iy0'] + 2 * bx['oyc'] - 1:2,
                              bx['p'], bx['zs']:bx['zs'] + 16]
                out_ap = pq[bank][p0:p1, h0:h0 + nh, bx['oy0']:bx['oy0'] + bx['oyc'], :]
                nc.tensor.matmul(out_ap, w[:, wc:wc + ww], rhs,
                                 start=False, stop=n in _STOPS)

            for n in range(N_C2):
                emit(n, _PLAN[n])

            # early evacuation of chunk2-only banks 5-7, overlapped with the
            # chunk-1 matmuls: LB -> osb planes 2..7, UB -> usb planes 0..5
            osb = outp.tile([CO, OXC, 16, 16], f32, tag="osb")
            usb = outp.tile([128, OXC, 16, 16], f32, tag="usb")
            for q in range(3):
                nc.vector.tensor_copy(osb[:, 2 + 2 * q:4 + 2 * q], pq[5 + q][0:64])
                nc.vector.tensor_copy(usb[64:128, 2 * q:2 * q + 2], pq[5 + q][64:128])

            for n in range(N_C2, len(_PLAN)):
                emit(n, _PLAN[n])

            # late evacuation: plane j needs L(j) [slots 0..7], U(j+1) for
            # j<=6 [slots 1..7], and plane 7 the edge partials from slot 8 L.
            # usb (U halves) first: the shift matmuls depend only on it, so
            # they overlap the osb ops that follow.
            nc.vector.tensor_add(usb[64:128, 0], usb[64:128, 0], pq[0][64:128, 1])
            nc.vector.tensor_add(usb[64:128, 1:3], usb[64:128, 1:3], pq[1][64:128])
            nc.vector.tensor_add(usb[64:128, 3:5], usb[64:128, 3:5], pq[2][64:128])
            nc.vector.tensor_add(usb[64:128, 5], usb[64:128, 5], pq[3][64:128, 0])
            nc.vector.tensor_copy(usb[64:128, 6], pq[3][64:128, 1])
            nc.vector.tensor_copy(osb[:, 0:2], pq[0][0:64])
            for m in range(1, 4):
                nc.vector.tensor_add(osb[:, 2 * m:2 * m + 2], osb[:, 2 * m:2 * m + 2],
                                     pq[m][0:64])
            nc.vector.tensor_add(osb[:, 7], osb[:, 7], pq[4][0:64, 0])

            # move upper-half partials (planes 0..6) down to partitions 0:64
            # via identity matmuls through the (now free) psum banks 0-3 —
            # no DRAM round-trip — then add into osb
            ident = aux_t[64:128, 64:128]
            for m in range(3):
                nc.tensor.matmul(pq[m][0:64], ident,
                                 usb[64:128, 2 * m:2 * m + 2],
                                 start=True, stop=True)
            nc.tensor.matmul(pq[3][0:64, 0], ident, usb[64:128, 6],
                             start=True, stop=True)
            for m in range(3):
                nc.vector.tensor_add(osb[:, 2 * m:2 * m + 2],
                                     osb[:, 2 * m:2 * m + 2], pq[m][0:64])
            nc.vector.tensor_add(osb[:, 6], osb[:, 6], pq[3][0:64, 0])

            # per-channel sum of squares in one fused scalar-engine op
            # (square + per-partition accumulate), then one matmul with the
            # combining matrix C turns it into the local variance [64, 1]
            of = osb.rearrange("c x y z -> c (x y z)")
            sq = usb.rearrange("c x y z -> c (x y z)")[0:64, :]
            ssq = stat.tile([CO, 1], f32, tag="ssq")
            ssq_b = stat.tile([CO, 1], f32, tag="ssqb")
            nc.scalar.activation(sq[:, 0:1024], of[:, 0:1024],
                                 mybir.ActivationFunctionType.Square,
                                 accum_out=ssq[:])
            nc.vector.tensor_mul(sq[:, 1024:2048], of[:, 1024:2048],
                                 of[:, 1024:2048])
            nc.vector.tensor_reduce(ssq_b[:], sq[:, 1024:2048],
                                    axis=mybir.AxisListType.X,
                                    op=mybir.AluOpType.add)
            nc.vector.tensor_add(ssq[:], ssq[:], ssq_b[:])
            nc.tensor.matmul(pq[5][0:64, 0, 0, 0:1], aux_t[0:64, 0:64],
                             ssq[:, :], start=True, stop=True)
            vcol = stat.tile([CO, 1], f32, tag="vcol")
            nc.vector.tensor_copy(vcol[:], pq[5][0:64, 0, 0, 0:1])

            v_dram = dram.tile([1, 64], f32, tag="vd")
            v_red = dram.tile([1, 64], f32, tag="vr")
            nc.sync.dma_start(v_dram[0, :], vcol[:, 0])
            nc.gpsimd.collective_compute(
                "AllReduce", mybir.AluOpType.add,
                replica_groups=[list(range(NCORES))],
                ins=[v_dram.opt()], outs=[v_red.opt()],
            )

            # scale = gamma / sqrt(var + eps), in per-partition layout
            var_col = stat.tile([CO, 1], f32, tag="varcol")
            nc.sync.dma_start(var_col[:, 0], v_red[0, :])
            nc.scalar.activation(sd[:], var_col[:], mybir.ActivationFunctionType.Sqrt,
                                 bias=eps_t[:], scale=1.0)
            inv = stat.tile([CO, 1], f32, tag="inv")
            nc.vector.reciprocal(inv[:], sd[:])
            gam_t = stat.tile([CO, 1], f32, tag="gam")
            nc.sync.dma_start(gam_t[:], gam_in[:])
            scale_col = stat.tile([CO, 1], f32, tag="sccol")
            nc.vector.tensor_mul(scale_col[:], inv[:], gam_t[:])
            bias_t = stat.tile([CO, 1], f32, tag="bias")
            nc.sync.dma_start(bias_t[:], bias_in[:])

            # BN scale everywhere (DVE partition ranges must be 32-aligned,
            # so no scalar/vector split), then bias+relu on scalar channels;
            # the vector-channel store only waits on the scale
            nc.vector.tensor_scalar_mul(of[:, :], of[:, :], scale_col[:, :])
            nc.scalar.activation(of[0:16, :], of[0:16, :],
                                 mybir.ActivationFunctionType.Relu,
                                 bias=bias_t[0:16, :], scale=1.0)
            # vector-channel store needs only the scale; it rides sync so it
            # overlaps the scalar-engine relu, whose store follows on scalar
            nc.sync.dma_start(out_d[16:64], osb[16:64])
            nc.scalar.dma_start(out_d[0:16], osb[0:16])

    nc.compile()
    return nc


def _install_ntff_hook():
    import sys, types
    if "antenv.axon_hooks" in sys.modules:
        return
    mod = types.ModuleType("antenv.axon_hooks")
    mod._hook = None
    mod.set_axon_ntff_profile_hook = lambda h: setattr(mod, "_hook", h)
    mod.get_axon_ntff_profile_hook = lambda: mod._hook
    sys.modules["antenv.axon_hooks"] = mod
    try:
        import antenv
        antenv.axon_hooks = mod
        from trn_agent_boot.trn_boot import _ntff_profile_via_ctypes
        mod.set_axon_ntff_profile_hook(_ntff_profile_via_ctypes("/opt/axon/libaxon_pjrt.so"))
    except Exception:
        pass


def run_on_hw(inp, trace=False):
    """Run the kernel on 8 cores. Returns (full output [4,64,16,16,16], results)."""
    from concourse.bass_utils import run_bass_kernel_spmd

    if "nc" not in _CACHED:
        _install_ntff_hook()
        _CACHED["nc"] = _build_bass()
    nc = _CACHED["nc"]

    svt = _svt_sym(inp['sv'])
    Ks = _assemble_kernel_sym(inp)
    WA, W2 = _weight_slabs(Ks)
    gam, bias = _gam_bias(inp['bn_g_s'], inp['bn_g_v'], inp['bias_s'])

    in_maps = []
    for c in range(NCORES):
        b, h = c // 2, c % 2
        c1, c2 = _core_slabs(svt, b, h)
        in_maps.append({
            "in1": c1,
            "in2": c2,
            "wa_in": WA,
            "w2_in": np.ascontiguousarray(W2.transpose(1, 0, 2).reshape(128, 49 * W2_COLS)),
            "aux_in": _aux_mat(),
            "gam_in": gam.reshape(64, 1),
            "bias_in": bias.reshape(64, 1),
        })

    res = run_bass_kernel_spmd(nc, in_maps, core_ids=list(range(NCORES)), trace=trace)

    y = np.zeros((B, CO, 16, 16, 16), np.float32)
    for c in range(NCORES):
        b, h = c // 2, c % 2
        y[b, :, 8 * h:8 * h + 8] = res.results[c]["out"]
    return y, res


def kernel(**inputs) -> np.ndarray:
    y, _ = run_on_hw(inputs, trace=False)
    return y
